# revision 17
# baseline (speedup 1.0000x reference)
"""Trainium2 Bass kernel for a spatial-attention block (AttentionBlock).

Shapes (hardcoded):
  x:  [B=8, C=128, W=64, H=64]   -> per core: x_b [128, 4096] fp32
  Wq: [A=16, C=128], bq: [16]
  Wk: [A=16, C=128], bk: [16]
  Wv: [C=128, C=128], bv: [128]
  out: [B, C, W, H]

Per-core computation (batch-parallel over 8 cores, no collectives):
  q = Wq x + bq           [16, 4096]
  k = Wk x + bk           [16, 4096]
  v = Wv x + bv           [128, 4096]
  S[i, j] = q[:, i] . k[:, j]
  P = softmax(S * 0.25, axis=j)
  out[:, i] = sum_j P[i, j] v[:, j]

Key layout trick: scores are computed TRANSPOSED (S_T[j, i], keys on
partitions) so the softmax reduction over keys j becomes a TensorE matmul
with a ones vector, and P feeds the PV matmul directly -- no transposes of
the [4096, 4096] attention matrix anywhere.  bv is added at the very end
(softmax rows sum to one, so  P (v + bv 1^T) = P v + bv).
"""

import os
import numpy as np

import concourse.bass as bass
import concourse.bacc as bacc
import concourse.mybir as mybir
import concourse.tile as tile
from concourse.bass_utils import run_bass_kernel_spmd

P = 128          # partitions / channels C
N = 4096         # tokens (64*64)
A = 16           # q/k head dim
NCORES = 8
IC = 1024        # i-chunk (query columns per chunk)
N_IC = N // IC   # 4
NJB = N // P     # 32 key blocks
EXP_SHIFT = 6.0  # subtracted inside exp for fp16 range safety
SCALE = 0.25     # 1/sqrt(A)

F32 = mybir.dt.float32
if os.environ.get("ATTN_BF16", ""):
    F16 = mybir.dt.bfloat16
    NP16 = "bfloat16"
else:
    F16 = mybir.dt.float16
    NP16 = "float16"
import ml_dtypes
NP16 = np.dtype(ml_dtypes.bfloat16) if NP16 == "bfloat16" else np.dtype(np.float16)

MM_N = int(os.environ.get("ATTN_MM_N", "512"))

# bisection flags
ROW_PACK = os.environ.get("ATTN_NO_ROWPACK", "") == ""   # 2-way PE row packing for S_T
GPSIMD_BCAST = os.environ.get("ATTN_NO_GPSIMD", "") == ""  # partition_broadcast vs DMA doubling

_CACHE = {}


def build_nc():
    nc = bacc.Bacc("TRN2", target_bir_lowering=False, name="attn_block")

    x_d = nc.dram_tensor("x", [P, N], F16, kind="ExternalInput")
    wqT_d = nc.dram_tensor("wqT", [P, A], F16, kind="ExternalInput")
    wkT_d = nc.dram_tensor("wkT", [P, A], F16, kind="ExternalInput")
    wvT_d = nc.dram_tensor("wvT", [P, P], F16, kind="ExternalInput")
    bq_d = nc.dram_tensor("bq", [A, 1], F32, kind="ExternalInput")
    bk_d = nc.dram_tensor("bk", [A, 1], F32, kind="ExternalInput")
    bv_d = nc.dram_tensor("bv", [P, 1], F32, kind="ExternalInput")
    out_d = nc.dram_tensor("out", [P, N], F32, kind="ExternalOutput")

    Exp = mybir.ActivationFunctionType.Exp
    Identity = mybir.ActivationFunctionType.Identity

    with tile.TileContext(nc) as tc:
        with (
            tc.tile_pool(name="const", bufs=1) as cpool,
            tc.tile_pool(name="work", bufs=1) as wpool,
        ):
            # ---- persistent SBUF tensors -------------------------------
            x_sb = wpool.tile([P, N], F16, tag="x")
            # q/k live on partitions 0:16 (row group 0) and replicated on
            # 32:48 (row group 1) for 2-way PE row packing of the S_T mms.
            q_sb = wpool.tile([48, N], F16, tag="q")
            k_sb = wpool.tile([48, N], F16, tag="k")
            v_sb = wpool.tile([P, NJB, P], F16, tag="v")  # [j, jb, v]

            wqT_sb = cpool.tile([P, A], F16, tag="wqT")
            wkT_sb = cpool.tile([P, A], F16, tag="wkT")
            wvT_sb = cpool.tile([P, P], F16, tag="wvT")
            bq_sb = cpool.tile([A, 1], F32, tag="bq")
            bk_sb = cpool.tile([A, 1], F32, tag="bk")
            bv_sb = cpool.tile([P, 1], F32, tag="bv")
            ones_sb = cpool.tile([P, 1], F16, tag="ones")
            eshift_sb = cpool.tile([P, 1], F32, tag="eshift")

            nc.sync.dma_start(x_sb[:], x_d[:])
            nc.sync.dma_start(wqT_sb[:], wqT_d[:])
            nc.sync.dma_start(wkT_sb[:], wkT_d[:])
            nc.sync.dma_start(wvT_sb[:], wvT_d[:])
            nc.sync.dma_start(bq_sb[:], bq_d[:])
            nc.sync.dma_start(bk_sb[:], bk_d[:])
            nc.sync.dma_start(bv_sb[:], bv_d[:])
            nc.vector.memset(ones_sb[:], 1.0)
            nc.vector.memset(eshift_sb[:], -EXP_SHIFT)

            # ---- phase 0: projections ----------------------------------
            with tc.tile_pool(name="psum0", bufs=2, space="PSUM") as pp0:
                # q = Wq x + bq, k = Wk x + bk   (fp32, [16, N])
                for h in range(N // 512):
                    sl = slice(h * 512, (h + 1) * 512)
                    qp = pp0.tile([A, 512], F32, tag="qk")
                    nc.tensor.matmul(qp[:], wqT_sb[:], x_sb[:, sl],
                                     start=True, stop=True)
                    nc.scalar.activation(q_sb[0:A, sl], qp[:], Identity,
                                         bias=bq_sb[:], scale=1.0)
                    kp = pp0.tile([A, 512], F32, tag="qk")
                    nc.tensor.matmul(kp[:], wkT_sb[:], x_sb[:, sl],
                                     start=True, stop=True)
                    nc.scalar.activation(k_sb[0:A, sl], kp[:], Identity,
                                         bias=bk_sb[:], scale=1.0)

                if ROW_PACK:
                    # replicate q/k to partitions 32:48 for row group 1.
                    # Emitted BEFORE the V_T matmuls so the DMA overlaps
                    # PE work -- a PE idle gap here re-throttles the HAM
                    # clock gate for the whole main loop.
                    nc.sync.dma_start(q_sb[32:48, :], q_sb[0:16, :])
                    nc.sync.dma_start(k_sb[32:48, :], k_sb[0:16, :])

                # V_T[j, v] = sum_c x[c, j] WvT[c, v]   (fp16 in SBUF)
                for g in range(NJB // 4):
                    vp = pp0.tile([P, 4, P], F32, tag="vt")
                    for r in range(4):
                        jb = 4 * g + r
                        nc.tensor.matmul(
                            vp[:, r, :],
                            x_sb[:, jb * P:(jb + 1) * P],
                            wvT_sb[:],
                            start=True, stop=True)
                    nc.vector.tensor_copy(v_sb[:, 4 * g:4 * g + 4, :], vp[:])

            # ---- main loop ---------------------------------------------
            with (
                tc.tile_pool(name="spsum", bufs=3, space="PSUM") as spool,
                tc.tile_pool(name="opsum", bufs=1, space="PSUM") as opool,
                tc.tile_pool(name="pexp", bufs=5) as ppool,
                tc.tile_pool(name="tree", bufs=7) as tpool,
                tc.tile_pool(name="fin", bufs=2) as fpool,
                tc.tile_pool(name="finsm", bufs=2) as fspool,
            ):
                for ic in range(N_IC):
                    isl = slice(ic * IC, (ic + 1) * IC)
                    o_ps = opool.tile([P, IC], F32, tag="o")

                    def st_mm(jb):
                        # S_T[jb] = k[:, jb].T @ q  (keys on partitions)
                        r = (jb % 2) if ROW_PACK else 0
                        pb = 32 * r
                        s_ps = spool.tile([P, IC], F32, tag="s",
                                          name=f"s_{ic}_{jb}")
                        for h in range(IC // MM_N):
                            hsl = slice(h * MM_N, (h + 1) * MM_N)
                            nc.tensor.matmul(
                                s_ps[:, hsl],
                                k_sb[pb:pb + A, jb * P:(jb + 1) * P],
                                q_sb[pb:pb + A,
                                     ic * IC + h * MM_N: ic * IC + (h + 1) * MM_N],
                                start=True, stop=True,
                                tile_position=(pb, 0))
                        return s_ps

                    # software pipeline: the PE queue is in-order, so queue
                    # the independent S_T of jb+1/jb+2 BEFORE the
                    # exp-dependent O matmuls of jb, letting PE run during
                    # exp(jb).
                    s_tiles = {0: st_mm(0), 1: st_mm(1)}

                    # Z accumulation: binary-counter tree of tensor_adds on
                    # VectorE (log-depth keeps 16-bit rounding error ~0.3%),
                    # finished by a single ones-matmul per chunk.
                    tree = []  # stack of (rank, tile)

                    def tree_push(t, rank=0):
                        while tree and tree[-1][0] == rank:
                            rank, prev = tree.pop()
                            nt = tpool.tile([P, IC], F16, tag="tree")
                            nc.vector.tensor_add(nt[:], prev[:], t[:])
                            t = nt
                            rank += 1
                        tree.append((rank, t))

                    for jb in range(NJB):
                        # P~ = exp(S*0.25 - 6)  (16-bit)
                        p_sb = ppool.tile([P, IC], F16, tag="p")
                        nc.scalar.activation(p_sb[:], s_tiles.pop(jb)[:], Exp,
                                             bias=eshift_sb[:], scale=SCALE)
                        if jb + 2 < NJB:
                            s_tiles[jb + 2] = st_mm(jb + 2)
                        # O += V_T[jb]^T @ P~
                        for h in range(IC // MM_N):
                            hsl = slice(h * MM_N, (h + 1) * MM_N)
                            nc.tensor.matmul(
                                o_ps[:, hsl],
                                v_sb[:, jb, :],
                                p_sb[:, hsl],
                                start=(jb == 0), stop=(jb == NJB - 1))
                        tree_push(p_sb)

                    assert len(tree) == 1, [r for r, _ in tree]
                    zroot = tree.pop()[1]
                    # Z = 1^T @ zroot  (exact fp32 reduction over partitions)
                    z_ps = spool.tile([1, IC], F32, tag="s", name=f"z_{ic}")
                    for h in range(IC // MM_N):
                        hsl = slice(h * MM_N, (h + 1) * MM_N)
                        nc.tensor.matmul(
                            z_ps[:, hsl], ones_sb[:], zroot[:, hsl],
                            start=True, stop=True)

                    # ---- chunk epilogue: out = O / Z + bv --------------
                    o_sb = fpool.tile([P, IC], F32, tag="osb")
                    nc.vector.tensor_copy(o_sb[:], o_ps[:])
                    rz = fspool.tile([1, IC], F32, tag="rz")
                    rscr = fspool.tile([1, IC], F32, tag="rscr")
                    nc.vector.reciprocal_approx_accurate(rz[:], z_ps[:], rscr[:])
                    rb = fpool.tile([P, IC], F32, tag="rb")
                    if GPSIMD_BCAST:
                        nc.gpsimd.partition_broadcast(rb[:], rz[:])
                    else:
                        nc.sync.dma_start(rb[0:1, :], rz[:])
                        p2 = 1
                        while p2 < P:
                            nc.sync.dma_start(rb[p2:2 * p2, :], rb[0:p2, :])
                            p2 *= 2
                    nc.vector.tensor_mul(o_sb[:], o_sb[:], rb[:])
                    nc.vector.tensor_scalar_add(o_sb[:], o_sb[:], bv_sb[:])
                    nc.sync.dma_start(out_d[:, isl], o_sb[:])

    nc.compile()
    return nc


def _get_program():
    if "nc" not in _CACHE:
        _CACHE["nc"] = build_nc()
    return _CACHE["nc"]


def kernel(x, Wq, bq, Wk, bk, Wv, bv, trace=False):
    x = np.ascontiguousarray(np.asarray(x).astype(NP16))
    Wq = np.asarray(Wq, dtype=np.float32)
    Wk = np.asarray(Wk, dtype=np.float32)
    Wv = np.asarray(Wv, dtype=np.float32)
    bq = np.asarray(bq, dtype=np.float32)
    bk = np.asarray(bk, dtype=np.float32)
    bv = np.asarray(bv, dtype=np.float32)

    B = x.shape[0]
    assert x.shape == (NCORES, P, 64, 64), x.shape

    wqT = np.ascontiguousarray(Wq.T.astype(NP16))   # [128, 16]
    wkT = np.ascontiguousarray(Wk.T.astype(NP16))   # [128, 16]
    wvT = np.ascontiguousarray(Wv.T.astype(NP16))   # [128, 128]
    bq2 = np.ascontiguousarray(bq.reshape(A, 1))
    bk2 = np.ascontiguousarray(bk.reshape(A, 1))
    bv2 = np.ascontiguousarray(bv.reshape(P, 1))

    nc = _get_program()
    in_maps = []
    for b in range(B):
        in_maps.append({
            "x": np.ascontiguousarray(x[b].reshape(P, N)),
            "wqT": wqT, "wkT": wkT, "wvT": wvT,
            "bq": bq2, "bk": bk2, "bv": bv2,
        })

    res = run_bass_kernel_spmd(nc, in_maps, list(range(NCORES)), trace=trace)
    out = np.stack([res.results[b]["out"].reshape(P, 64, 64) for b in range(B)])
    if trace:
        _CACHE["last_results"] = res
    return out


# revision 24
# speedup vs baseline: 1.2415x; 1.2415x over previous
"""Trainium2 Bass kernel for a spatial-attention block (AttentionBlock).

Shapes (hardcoded):
  x:  [B=8, C=128, W=64, H=64]   -> per core: x_b [128, 4096] fp32
  Wq: [A=16, C=128], bq: [16]
  Wk: [A=16, C=128], bk: [16]
  Wv: [C=128, C=128], bv: [128]
  out: [B, C, W, H]

Per-core computation (batch-parallel over 8 cores, no collectives):
  q = Wq x + bq           [16, 4096]
  k = Wk x + bk           [16, 4096]
  v = Wv x + bv           [128, 4096]
  S[i, j] = q[:, i] . k[:, j]
  P = softmax(S * 0.25, axis=j)
  out[:, i] = sum_j P[i, j] v[:, j]

Key layout trick: scores are computed TRANSPOSED (S_T[j, i], keys on
partitions) so the softmax reduction over keys j becomes a TensorE matmul
with a ones vector, and P feeds the PV matmul directly -- no transposes of
the [4096, 4096] attention matrix anywhere.  bv is added at the very end
(softmax rows sum to one, so  P (v + bv 1^T) = P v + bv).
"""

import os
import numpy as np

import concourse.bass as bass
import concourse.bacc as bacc
import concourse.mybir as mybir
import concourse.tile as tile
from concourse.bass_utils import run_bass_kernel_spmd

P = 128          # partitions / channels C
N = 4096         # tokens (64*64)
A = 16           # q/k head dim
NCORES = 8
IC = 1024        # i-chunk (query columns per chunk)
N_IC = N // IC   # 4
NJB = N // P     # 32 key blocks
EXP_SHIFT = 6.0  # subtracted inside exp for fp16 range safety
SCALE = 0.25     # 1/sqrt(A)

F32 = mybir.dt.float32
if os.environ.get("ATTN_BF16", ""):
    F16 = mybir.dt.bfloat16
    NP16 = "bfloat16"
else:
    F16 = mybir.dt.float16
    NP16 = "float16"
import ml_dtypes
NP16 = np.dtype(ml_dtypes.bfloat16) if NP16 == "bfloat16" else np.dtype(np.float16)

MM_N = int(os.environ.get("ATTN_MM_N", "512"))

if os.environ.get("ATTN_LDW_OPT", ""):
    # walrus's LDWEIGHTS dedup pass is disabled by default in this harness;
    # re-enable it so consecutive same-weight matmuls skip the reload.
    from concourse import bass_utils as _bu
    if not getattr(_bu, "_ldw_opt_patched", False):
        _orig_run_command = _bu.run_command

        def _run_command_ldw(argv, **kw):
            argv = ["--enable-ldw-opt=true" if a == "--enable-ldw-opt=false"
                    else a for a in argv]
            return _orig_run_command(argv, **kw)

        _bu.run_command = _run_command_ldw
        _bu._ldw_opt_patched = True

# bisection flags
ROW_PACK = os.environ.get("ATTN_NO_ROWPACK", "") == ""   # 2-way PE row packing for S_T
GPSIMD_BCAST = os.environ.get("ATTN_NO_GPSIMD", "") == ""  # partition_broadcast vs DMA doubling
DUMMY_PER_JB = int(os.environ.get("ATTN_DUMMY", "2"))    # HAM warmth-keeper MMs per jb
WARMUP_MM = int(os.environ.get("ATTN_WARMUP", "20"))     # warm-up burst at main-loop start

_CACHE = {}


def build_nc():
    nc = bacc.Bacc("TRN2", target_bir_lowering=False, name="attn_block")

    x_d = nc.dram_tensor("x", [P, N], F16, kind="ExternalInput")
    dbg_d = nc.dram_tensor("dbg", [1, 512], F32, kind="ExternalOutput")
    wqT_d = nc.dram_tensor("wqT", [P, A], F16, kind="ExternalInput")
    wkT_d = nc.dram_tensor("wkT", [P, A], F16, kind="ExternalInput")
    wvT_d = nc.dram_tensor("wvT", [P, P], F16, kind="ExternalInput")
    bq_d = nc.dram_tensor("bq", [A, 1], F32, kind="ExternalInput")
    bk_d = nc.dram_tensor("bk", [A, 1], F32, kind="ExternalInput")
    bv_d = nc.dram_tensor("bv", [P, 1], F32, kind="ExternalInput")
    out_d = nc.dram_tensor("out", [P, N], F32, kind="ExternalOutput")

    Exp = mybir.ActivationFunctionType.Exp
    Identity = mybir.ActivationFunctionType.Identity

    with tile.TileContext(nc) as tc:
        with (
            tc.tile_pool(name="const", bufs=1) as cpool,
            tc.tile_pool(name="work", bufs=1) as wpool,
        ):
            # ---- persistent SBUF tensors -------------------------------
            x_sb = wpool.tile([P, N], F16, tag="x")
            # q/k live on partitions 0:16 (row group 0) and replicated on
            # 32:48 (row group 1) for 2-way PE row packing of the S_T mms.
            q_sb = wpool.tile([48, N], F16, tag="q")
            k_sb = wpool.tile([48, N], F16, tag="k")
            v_sb = wpool.tile([P, NJB, P], F16, tag="v")  # [j, jb, v]

            wqT_sb = cpool.tile([P, A], F16, tag="wqT")
            wkT_sb = cpool.tile([P, A], F16, tag="wkT")
            wvT_sb = cpool.tile([P, P], F16, tag="wvT")
            bq_sb = cpool.tile([A, 1], F32, tag="bq")
            bk_sb = cpool.tile([A, 1], F32, tag="bk")
            bv_sb = cpool.tile([P, 1], F32, tag="bv")
            ones_sb = cpool.tile([P, 1], F16, tag="ones")
            eshift_sb = cpool.tile([P, 1], F32, tag="eshift")

            nc.sync.dma_start(x_sb[:], x_d[:])
            nc.sync.dma_start(wqT_sb[:], wqT_d[:])
            nc.sync.dma_start(wkT_sb[:], wkT_d[:])
            nc.sync.dma_start(wvT_sb[:], wvT_d[:])
            nc.sync.dma_start(bq_sb[:], bq_d[:])
            nc.sync.dma_start(bk_sb[:], bk_d[:])
            nc.sync.dma_start(bv_sb[:], bv_d[:])
            nc.vector.memset(ones_sb[:], 1.0)
            nc.vector.memset(eshift_sb[:], -EXP_SHIFT)

            # ---- phase 0: projections ----------------------------------
            with tc.tile_pool(name="psum0", bufs=2, space="PSUM") as pp0:
                # q = Wq x + bq, k = Wk x + bk   (fp32, [16, N])
                for h in range(N // 512):
                    sl = slice(h * 512, (h + 1) * 512)
                    qp = pp0.tile([A, 512], F32, tag="qk")
                    nc.tensor.matmul(qp[:], wqT_sb[:], x_sb[:, sl],
                                     start=True, stop=True)
                    nc.scalar.activation(q_sb[0:A, sl], qp[:], Identity,
                                         bias=bq_sb[:], scale=1.0)
                    kp = pp0.tile([A, 512], F32, tag="qk")
                    nc.tensor.matmul(kp[:], wkT_sb[:], x_sb[:, sl],
                                     start=True, stop=True)
                    nc.scalar.activation(k_sb[0:A, sl], kp[:], Identity,
                                         bias=bk_sb[:], scale=1.0)

                if ROW_PACK:
                    # replicate q/k to partitions 32:48 for row group 1.
                    # Emitted BEFORE the V_T matmuls so the DMA overlaps
                    # PE work -- a PE idle gap here re-throttles the HAM
                    # clock gate for the whole main loop.
                    nc.sync.dma_start(q_sb[32:48, :], q_sb[0:16, :])
                    nc.sync.dma_start(k_sb[32:48, :], k_sb[0:16, :])

                # V_T[j, v] = sum_c x[c, j] WvT[c, v]   (fp16 in SBUF)
                for g in range(NJB // 4):
                    vp = pp0.tile([P, 4, P], F32, tag="vt")
                    for r in range(4):
                        jb = 4 * g + r
                        nc.tensor.matmul(
                            vp[:, r, :],
                            x_sb[:, jb * P:(jb + 1) * P],
                            wvT_sb[:],
                            start=True, stop=True)
                    nc.vector.tensor_copy(v_sb[:, 4 * g:4 * g + 4, :], vp[:])

            # ---- main loop ---------------------------------------------
            with (
                tc.tile_pool(name="spsum", bufs=2, space="PSUM") as spool,
                tc.tile_pool(name="opsum", bufs=1, space="PSUM") as opool,
                tc.tile_pool(name="dummy", bufs=1, space="PSUM") as dpool,
                tc.tile_pool(name="pexp", bufs=5) as ppool,
                tc.tile_pool(name="tree", bufs=7) as tpool,
                tc.tile_pool(name="fin", bufs=2) as fpool,
                tc.tile_pool(name="finsm", bufs=2) as fspool,
            ):
                # Warmth-keeper: the PE HAM clock gate drops to 1.2 GHz
                # whenever PE activity dips and only re-arms on ~3.4us of
                # dense streaming.  A single long accumulation chain of
                # no-semaphore dummy matmuls (always ready, N=512 streams)
                # interleaved into the PE queue fills every dependency
                # stall so the array stays at 2.4 GHz.
                dummy_state = {"n": 0}
                dummy_ps = dpool.tile([1, 512], F32, tag="dummy",
                                      name="dummy_ps")

                def dummy_mm(count):
                    for _ in range(count):
                        nc.tensor.matmul(
                            dummy_ps[:], ones_sb[:], x_sb[0:P, 0:512],
                            start=(dummy_state["n"] == 0), stop=False,
                            skip_group_check=True)
                        dummy_state["n"] += 1

                # dense warm-up burst bridging the phase0 -> main transition
                dummy_mm(WARMUP_MM)

                for ic in range(N_IC):
                    isl = slice(ic * IC, (ic + 1) * IC)
                    o_ps = opool.tile([P, IC], F32, tag="o")

                    def st_mm(jb):
                        # S_T[jb] = k[:, jb].T @ q  (keys on partitions)
                        r = (jb % 2) if ROW_PACK else 0
                        pb = 32 * r
                        s_ps = spool.tile([P, IC], F32, tag="s",
                                          name=f"s_{ic}_{jb}")
                        for h in range(IC // MM_N):
                            hsl = slice(h * MM_N, (h + 1) * MM_N)
                            nc.tensor.matmul(
                                s_ps[:, hsl],
                                k_sb[pb:pb + A, jb * P:(jb + 1) * P],
                                q_sb[pb:pb + A,
                                     ic * IC + h * MM_N: ic * IC + (h + 1) * MM_N],
                                start=True, stop=True,
                                tile_position=(pb, 0))
                        return s_ps

                    # software pipeline: the PE queue is in-order, so queue
                    # the independent S_T of jb+1/jb+2 BEFORE the
                    # exp-dependent O matmuls of jb, letting PE run during
                    # exp(jb).
                    s_tiles = {0: st_mm(0), 1: st_mm(1)}

                    # Z accumulation: binary-counter tree of tensor_adds on
                    # VectorE (log-depth keeps 16-bit rounding error ~0.3%),
                    # finished by a single ones-matmul per chunk.
                    tree = []  # stack of (rank, tile)

                    def tree_push(t, rank=0):
                        while tree and tree[-1][0] == rank:
                            rank, prev = tree.pop()
                            nt = tpool.tile([P, IC], F16, tag="tree")
                            nc.vector.tensor_add(nt[:], prev[:], t[:])
                            t = nt
                            rank += 1
                        tree.append((rank, t))

                    for jb in range(NJB):
                        # P~ = exp(S*0.25 - 6)  (16-bit)
                        p_sb = ppool.tile([P, IC], F16, tag="p")
                        nc.scalar.activation(p_sb[:], s_tiles.pop(jb)[:], Exp,
                                             bias=eshift_sb[:], scale=SCALE)
                        if jb + 2 < NJB:
                            s_tiles[jb + 2] = st_mm(jb + 2)
                        dummy_mm(DUMMY_PER_JB)
                        # O += V_T[jb]^T @ P~
                        for h in range(IC // MM_N):
                            hsl = slice(h * MM_N, (h + 1) * MM_N)
                            nc.tensor.matmul(
                                o_ps[:, hsl],
                                v_sb[:, jb, :],
                                p_sb[:, hsl],
                                start=(jb == 0), stop=(jb == NJB - 1))
                        tree_push(p_sb)

                    assert len(tree) == 1, [r for r, _ in tree]
                    zroot = tree.pop()[1]
                    # Z = 1^T @ zroot  (exact fp32 reduction over partitions)
                    z_ps = spool.tile([1, IC], F32, tag="s", name=f"z_{ic}")
                    for h in range(IC // MM_N):
                        hsl = slice(h * MM_N, (h + 1) * MM_N)
                        nc.tensor.matmul(
                            z_ps[:, hsl], ones_sb[:], zroot[:, hsl],
                            start=True, stop=True)

                    # ---- chunk epilogue: out = O / Z + bv --------------
                    o_sb = fpool.tile([P, IC], F32, tag="osb")
                    nc.vector.tensor_copy(o_sb[:], o_ps[:])
                    rz = fspool.tile([1, IC], F32, tag="rz")
                    rscr = fspool.tile([1, IC], F32, tag="rscr")
                    nc.vector.reciprocal_approx_accurate(rz[:], z_ps[:], rscr[:])
                    rb = fpool.tile([P, IC], F32, tag="rb")
                    if GPSIMD_BCAST:
                        nc.gpsimd.partition_broadcast(rb[:], rz[:])
                    else:
                        nc.sync.dma_start(rb[0:1, :], rz[:])
                        p2 = 1
                        while p2 < P:
                            nc.sync.dma_start(rb[p2:2 * p2, :], rb[0:p2, :])
                            p2 *= 2
                    nc.vector.tensor_mul(o_sb[:], o_sb[:], rb[:])
                    nc.vector.tensor_scalar_add(o_sb[:], o_sb[:], bv_sb[:])
                    nc.sync.dma_start(out_d[:, isl], o_sb[:])

                # close the dummy accumulation group and make it reachable
                nc.tensor.matmul(
                    dummy_ps[:], ones_sb[:], x_sb[0:P, 0:512],
                    start=False, stop=True, skip_group_check=True)
                dbg_sb = fspool.tile([1, 512], F32, tag="dbg")
                nc.vector.tensor_copy(dbg_sb[:], dummy_ps[:])
                nc.sync.dma_start(dbg_d[:], dbg_sb[:])

    nc.compile()
    return nc


def _get_program():
    if "nc" not in _CACHE:
        _CACHE["nc"] = build_nc()
    return _CACHE["nc"]


def kernel(x, Wq, bq, Wk, bk, Wv, bv, trace=False):
    x = np.ascontiguousarray(np.asarray(x).astype(NP16))
    Wq = np.asarray(Wq, dtype=np.float32)
    Wk = np.asarray(Wk, dtype=np.float32)
    Wv = np.asarray(Wv, dtype=np.float32)
    bq = np.asarray(bq, dtype=np.float32)
    bk = np.asarray(bk, dtype=np.float32)
    bv = np.asarray(bv, dtype=np.float32)

    B = x.shape[0]
    assert x.shape == (NCORES, P, 64, 64), x.shape

    wqT = np.ascontiguousarray(Wq.T.astype(NP16))   # [128, 16]
    wkT = np.ascontiguousarray(Wk.T.astype(NP16))   # [128, 16]
    wvT = np.ascontiguousarray(Wv.T.astype(NP16))   # [128, 128]
    bq2 = np.ascontiguousarray(bq.reshape(A, 1))
    bk2 = np.ascontiguousarray(bk.reshape(A, 1))
    bv2 = np.ascontiguousarray(bv.reshape(P, 1))

    nc = _get_program()
    in_maps = []
    for b in range(B):
        in_maps.append({
            "x": np.ascontiguousarray(x[b].reshape(P, N)),
            "wqT": wqT, "wkT": wkT, "wvT": wvT,
            "bq": bq2, "bk": bk2, "bv": bv2,
        })

    res = run_bass_kernel_spmd(nc, in_maps, list(range(NCORES)), trace=trace)
    out = np.stack([res.results[b]["out"].reshape(P, 64, 64) for b in range(B)])
    if trace:
        _CACHE["last_results"] = res
    return out
